# revision 2
# baseline (speedup 1.0000x reference)
"""Trainium2 Bass kernel for the global-context-fusion block.

Reference computation (per batch sample b):
    pooled[c] = mean_{h,w} x[b,c,h,w]                         # [C]
    y1 = relu6(w_guide @ pooled)                              # [R]
    y2 = relu6((w_fuse @ y1 - bn_mean) * inv_std * g + beta)  # [C]
    out[b,c,h,w] = x[b,c,h,w] + y2[c]

Strategy: data-parallel over batch — 8 samples, 8 NeuronCores, one sample per
core; the tiny 1x1-path params are replicated. Per core x is [512, 16384] f32
(32 MiB) and the kernel is HBM-bound (HBM-per-NC limit ~358 GB/s). HBM
traffic is the whole game: x must be consumed twice (pool, then broadcast
add) but SBUF can hold the full sample only at reduced precision. So pass 1
streams x in fp32, converts it to a fully SBUF-resident bf16 copy (16 MiB)
while accumulating the pool sums in fp32, and pass 2 adds y2 from the bf16
copy — no second read. Traffic is the 64 MiB floor (32 read + 32 write)
instead of the 80 MiB of a fp32-partial-cache scheme. bf16 rounding of x
adds ~1e-3 relative error against a 2e-2 budget.

Pass-1 conversion+reduce alternates between ScalarE (activation Copy with
accum_out does both in one pass) and DVE (copy then reduce); both together
run ~3x faster than the DMA stream, so conversion never gates the reads.
The last tile of the last chunk is split across both engines so the
pool->y1->y2 barrier sees only ~1 us of conversion tail. y1's four
accumulating matmuls are issued per-chunk as soon as each chunk's sum
column is ready, so at the barrier only chunk 3's reduce + one matmul +
the tiny y2 chain remain.

Host-side folding (all on tiny [C]-sized tensors):
    wg = (w_guide / HW).T          -> pool division folded into first matmul
    wf = (w_fuse * bn_scale).T     -> BN scale folded into second matmul
    b2 = beta - mean * bn_scale    -> BN shift applied as bias before relu6
"""

import numpy as np

from concourse import bass, mybir, tile
from concourse.bass_utils import run_bass_kernel_spmd

# Problem shapes (nn_GCF_FPGA_68032281969033), hardcoded per harness contract.
B, C, H, W = 8, 512, 128, 128
HW = H * W
R = 128
P = 128
BN_EPS = 1e-5

M_CHUNKS = C // P        # channel chunks of 128 partitions
F = 2048                 # stream tile width (1 MiB fp32 per DMA)
J = HW // F              # F-subtiles per channel chunk
HALF = F // 2
W_BUFS = 8               # shared landing (pass 1) / staging (pass 2) slots

# Partial-sum column layout: chunks 0-2 use 8 cols each (24); chunk 3 uses
# cols 24-30 for subtiles 0-6, col 31 for their pre-reduce, cols 32-33 for
# the split last tile. Final chunk-3 sum reduces the contiguous cols 31-34.
N_PART = 34

FP32 = mybir.dt.float32
BF16 = mybir.dt.bfloat16
AX = mybir.AxisListType.X
ALU = mybir.AluOpType
ACTF = mybir.ActivationFunctionType


def _build_program() -> bass.Bass:
    nc = bass.Bass()
    x_d = nc.declare_dram_parameter("x", [C, HW], FP32, isOutput=False)
    wg_d = nc.declare_dram_parameter("wg", [C, R], FP32, isOutput=False)
    wf_d = nc.declare_dram_parameter("wf", [R, C], FP32, isOutput=False)
    # b2 padded to 512 B lines per partition: sub-512 B DMA lines pay the SDMA
    # read-modify-write penalty and stall the ring head.
    b2_d = nc.declare_dram_parameter("b2", [P, 128], FP32, isOutput=False)
    out_d = nc.declare_dram_parameter("out", [C, HW], FP32, isOutput=True)

    with tile.TileContext(nc) as tc:
        with (
            tc.tile_pool(name="params", bufs=1) as ppool,
            tc.tile_pool(name="cache", bufs=1) as cpool,
            tc.tile_pool(name="work", bufs=W_BUFS) as wpool,
            tc.tile_pool(name="psum", bufs=1, space="PSUM") as qpool,
        ):
            # Params at the head of the SP ring: they are small and drain in a
            # couple of microseconds before the bulk x-loads start.
            wg_raw = ppool.tile([P, M_CHUNKS, R], FP32, tag="wg_raw")
            nc.sync.dma_start(out=wg_raw[:], in_=wg_d.rearrange("(k p) r -> p k r", p=P))
            wf_raw = ppool.tile([P, C], FP32, tag="wf_raw")
            nc.sync.dma_start(out=wf_raw[:], in_=wf_d[:])
            b2_t = ppool.tile([P, 128], FP32, tag="b2")
            nc.sync.dma_start(out=b2_t[:], in_=b2_d[:])

            # Matmul (LDWEIGHTS) instructions only get one sync-wait slot in
            # walrus codegen, but they read both DMA-landed weights and
            # DVE-produced activations. Staging the weights through a DVE copy
            # makes every matmul input DVE-produced -> a single DVE wait.
            wg_t = ppool.tile([P, M_CHUNKS, R], FP32, tag="wg")
            nc.vector.tensor_copy(out=wg_t[:], in_=wg_raw[:])
            wf_t = ppool.tile([P, C], FP32, tag="wf")
            nc.vector.tensor_copy(out=wf_t[:], in_=wf_raw[:])

            part_t = ppool.tile([P, N_PART], FP32, tag="part")
            sums_t = ppool.tile([P, M_CHUNKS], FP32, tag="sums")
            y1_t = ppool.tile([P, 1], FP32, tag="y1")
            y2_t = ppool.tile([P, M_CHUNKS], FP32, tag="y2")

            cache = [cpool.tile([P, HW], BF16, tag=f"c{m}", name=f"c{m}")
                     for m in range(M_CHUNKS)]

            p1 = qpool.tile([P, 1], FP32, tag="p1")

            def conv_scalar(src, dst, col):
                # fp32 -> bf16 copy + fp32 row-sum in one ScalarE pass
                nc.scalar.activation(
                    out=dst, in_=src, func=ACTF.Copy,
                    accum_out=part_t[:, col : col + 1],
                )

            def conv_vector(src, dst, col):
                nc.vector.tensor_copy(out=dst, in_=src)
                nc.vector.reduce_sum(
                    out=part_t[:, col : col + 1], in_=dst, axis=AX
                )

            # Pass 1: stream x, convert to resident bf16, accumulate pool sums.
            for m in range(M_CHUNKS):
                lo = m * J
                for j in range(J):
                    last = (m == M_CHUNKS - 1) and (j == J - 1)
                    t = wpool.tile([P, F], FP32, tag="w", name="t")
                    nc.sync.dma_start(
                        out=t[:], in_=x_d[m * P : (m + 1) * P, j * F : (j + 1) * F]
                    )
                    if not last:
                        dst = cache[m][:, j * F : (j + 1) * F]
                        if j % 2 == 0:
                            conv_scalar(t[:], dst, lo + j)
                        else:
                            conv_vector(t[:], dst, lo + j)
                    else:
                        # Pre-reduce this chunk's earlier partials while the
                        # last tile is still in flight, then split the last
                        # tile's conversion across both engines to shorten
                        # the barrier tail.
                        nc.vector.reduce_sum(
                            out=part_t[:, 31:32], in_=part_t[:, lo : lo + J - 1],
                            axis=AX,
                        )
                        conv_scalar(t[:, :HALF], cache[m][:, j * F : j * F + HALF], 32)
                        conv_vector(t[:, HALF:], cache[m][:, j * F + HALF : (j + 1) * F], 33)
                # Chunk sum + incremental y1 matmul (PE accumulates in PSUM).
                if m < M_CHUNKS - 1:
                    nc.vector.reduce_sum(
                        out=sums_t[:, m : m + 1], in_=part_t[:, lo : lo + J], axis=AX
                    )
                else:
                    nc.vector.reduce_sum(
                        out=sums_t[:, m : m + 1], in_=part_t[:, 31:34], axis=AX
                    )
                nc.tensor.matmul(
                    p1[:],
                    wg_t[:, m, :],
                    sums_t[:, m : m + 1],
                    start=(m == 0),
                    stop=(m == M_CHUNKS - 1),
                )

            # y1 = relu6(p1); y2 = relu6(wf.T @ y1 + b2).
            nc.vector.tensor_scalar(
                out=y1_t[:], in0=p1[:], scalar1=0.0, scalar2=6.0, op0=ALU.max, op1=ALU.min
            )
            p2 = qpool.tile([P, M_CHUNKS], FP32, tag="p2")
            for m in range(M_CHUNKS):
                nc.tensor.matmul(
                    p2[:, m : m + 1],
                    wf_t[:, m * P : (m + 1) * P],
                    y1_t[:],
                    start=True,
                    stop=True,
                )
            nc.vector.tensor_add(out=y2_t[:], in0=p2[:], in1=b2_t[:, :M_CHUNKS])
            nc.vector.tensor_scalar(
                out=y2_t[:], in0=y2_t[:], scalar1=0.0, scalar2=6.0, op0=ALU.max, op1=ALU.min
            )

            # Pass 2: out = bf16(x) + y2[channel], straight from SBUF.
            # First two tiles are half-width so the store stream starts as
            # soon after y2 as possible; adds alternate ScalarE/DVE so the
            # add stream runs ~2x ahead of the store DMAs.
            tiles2 = []
            for m in range(M_CHUNKS):
                widths = ([HALF, HALF] if m == 0 else [F]) + [F] * (J - 1)
                off = 0
                for w_ in widths:
                    tiles2.append((m, off, w_))
                    off += w_
            for k, (m, off, w_) in enumerate(tiles2):
                s = wpool.tile([P, F], FP32, tag="w", name="s")
                src = cache[m][:, off : off + w_]
                if k % 2 == 0:
                    nc.scalar.add(out=s[:, :w_], in_=src, add=y2_t[:, m : m + 1])
                else:
                    nc.vector.tensor_scalar_add(
                        out=s[:, :w_], in0=src, scalar1=y2_t[:, m : m + 1]
                    )
                nc.sync.dma_start(
                    out=out_d[m * P : (m + 1) * P, off : off + w_], in_=s[:, :w_]
                )

    _hoist_excess_waits(nc)
    return nc


# walrus codegen has per-instruction sync-wait slot limits (the Matmult
# LDWEIGHTS struct fits one wait; the DMA DIRECT2D struct fits two). Tile's
# sem assignment is not transitively minimal and can exceed them. Excess waits
# are hoisted into standalone EventSemaphore instructions placed right before
# the instruction on the same engine queue — identical semantics (inline DMA
# waits execute at the issuing sequencer too), just a different encoding.
_WAIT_CAPS = {
    "InstMatmult": 1,
    "InstActivation": 1,
    "InstDMACopy": 1,
    "InstTensorReduce": 1,
    "InstTensorScalarPtr": 1,
    "InstTensorTensor": 1,
    "InstTensorCopy": 1,
    "InstMemset": 1,
    "InstDrain": 1,
}


def _hoist_excess_waits(nc: bass.Bass) -> None:
    n = 0
    for bb in nc.main_func.blocks:
        il = bb.instructions
        new_list = []
        for ins in il:
            si = ins.sync_info
            cap = _WAIT_CAPS.get(type(ins).__name__)
            if si is not None and cap is not None and len(si.on_wait) > cap:
                waits = list(si.on_wait)
                for w in waits[cap:]:
                    n += 1
                    es = mybir.InstEventSemaphore(
                        name=f"I-hoistwait-{n}",
                        engine=ins.engine,
                        sync_info=mybir.SyncInfo(on_wait=[w], on_update=[]),
                    )
                    new_list.append(es)
                ins.sync_info = mybir.SyncInfo(
                    on_wait=waits[:cap], on_update=list(si.on_update)
                )
            new_list.append(ins)
        if len(new_list) != len(il):
            il[:] = new_list


_NC = None


def _get_nc() -> bass.Bass:
    global _NC
    if _NC is None:
        _NC = _build_program()
    return _NC


def _prep_in_maps(x, w_guide, w_fuse, bn_gamma, bn_beta, bn_mean, bn_var):
    x = np.asarray(x, dtype=np.float32)
    w_guide = np.asarray(w_guide, dtype=np.float32)
    w_fuse = np.asarray(w_fuse, dtype=np.float32)
    bn_gamma = np.asarray(bn_gamma, dtype=np.float32)
    bn_beta = np.asarray(bn_beta, dtype=np.float32)
    bn_mean = np.asarray(bn_mean, dtype=np.float32)
    bn_var = np.asarray(bn_var, dtype=np.float32)

    scale = bn_gamma / np.sqrt(bn_var + np.float32(BN_EPS))
    wg = np.ascontiguousarray((w_guide / np.float32(HW)).T)           # [C, R]
    wf = np.ascontiguousarray((w_fuse * scale[:, None]).T)            # [R, C]
    b2 = np.zeros((P, 128), dtype=np.float32)  # padded to 512 B DMA lines
    b2[:, :M_CHUNKS] = (bn_beta - bn_mean * scale).reshape(M_CHUNKS, P).T

    xs = np.ascontiguousarray(x.reshape(B, C, HW))
    return [{"x": xs[i], "wg": wg, "wf": wf, "b2": b2} for i in range(B)]


def run(inputs: dict, **kwargs):
    """Run the SPMD kernel; returns the BassKernelResults (for profiling)."""
    nc = _get_nc()
    in_maps = _prep_in_maps(**inputs)
    return run_bass_kernel_spmd(nc, in_maps, core_ids=list(range(B)), **kwargs)


def kernel(**inputs) -> np.ndarray:
    res = run(inputs)
    out = np.stack([np.asarray(res.results[i]["out"]) for i in range(B)], axis=0)
    return out.reshape(B, C, H, W).astype(np.float32, copy=False)
